# revision 1
# baseline (speedup 1.0000x reference)
"""HMM forward kernel v2 — time-segmented, latency-optimized.

Per core: 128 sequences x Ns=1024 steps, K=64 states.  Serial depth is cut
H-fold by splitting time into H segments: products of positive matrices
become rank-1 (Birkhoff contraction), so segment h>=1 only needs
  y_h = M_h @ 1      (forward chain from all-ones)
  w_h = M_h^T @ 1    (backward chain from all-ones)
and the total log-likelihood stitches with dot products:
  ll = C0 + sum_h Cf_h + ln(w_1.x0) + sum_h ln(w_h.y_{h-1}) + ln(1.y_{H-1})
       - sum_h ln(w_h.1)
(C* = logged renorm scales; backward scales cancel and are not logged.)

All 2H-1 chains advance in lockstep; ONE DVE tensor_tensor per slot does
every chain's elementwise P-multiply (amortizing the PSUM access penalty),
with two sequence-staggered groups (64 seqs each) to hide chain latency.

Layouts (per stagger-group gp):
  state  s   (128p, NCH, 32)  bf16   partition = 64*gs + k
  psum   v   (128p, NCH, 32)  f32    one bank
  chunk  P   (128p, NCH, 32*TCc) f32 per-c [b][t], exp'd in place
Chain order c: [seg0, fwd1..fwdH-1, bwd1..bwdH-1].
Backward chains use the pre-multiplied form w~_s = P_t(s) * (Tp @ w~_{s-1})
so every chain is matmul->multiply with the same slot alignment; host packs
backward P regions time-reversed.

Host packs "lp" as [gp][chunk i][gs][c][k][b][t] so every DMA is dense
2KB+ contiguous runs.
"""

from contextlib import ExitStack

import numpy as np
import ml_dtypes

import concourse.bass as bass
import concourse.tile as tile
from concourse import bacc, mybir

F32 = mybir.dt.float32
F16 = mybir.dt.float16
BF16 = mybir.dt.bfloat16
AFT = mybir.ActivationFunctionType

K = 64
CHAT = 0.5


def build_nc(ns=1024, h_seg=8, tc_chunk=16, r=64, lp_fp16=True,
             trn_type="TRN2"):
    S = ns // h_seg
    NCH = 2 * h_seg - 1
    n_chunks = S // tc_chunk
    nc = bacc.Bacc(trn_type, target_bir_lowering=False, debug=False)

    lp = nc.dram_tensor("lp", [2, n_chunks, NCH, 2, K, 32, tc_chunk],
                        F16 if lp_fp16 else F32, kind="ExternalInput")
    wts = nc.dram_tensor("wts", [3, 128, 128], BF16, kind="ExternalInput")
    cols = nc.dram_tensor("cols", [2, 128], F32, kind="ExternalInput")
    out_ll = nc.dram_tensor("ll", [2, 128, 32], F32, kind="ExternalOutput")

    with tile.TileContext(nc) as tc:
        with ExitStack() as ctx:
            _emit(ctx, tc, lp.ap(), wts.ap(), cols.ap(), out_ll.ap(),
                  S=S, H=h_seg, NCH=NCH, TCc=tc_chunk, n_chunks=n_chunks, r=r)
    nc.compile()
    return nc


def _emit(ctx, tc, lp, wts, cols, out_ll, *, S, H, NCH, TCc, n_chunks, r):
    nc = tc.nc
    BT = 32 * TCc

    consts = ctx.enter_context(tc.tile_pool(name="consts", bufs=1))
    pch_pools = [ctx.enter_context(tc.tile_pool(name=f"pch{g}", bufs=2))
                 for g in (0, 1)]
    s_pools = [ctx.enter_context(tc.tile_pool(name=f"s{g}", bufs=4))
               for g in (0, 1)]
    small = ctx.enter_context(tc.tile_pool(name="small", bufs=2))
    # v tile needs 2 PSUM banks when NCH>15; bufs=1 is safe (the WAR on the
    # bank coincides with the RAW chain through the state tile)
    vbufs = 3 if NCH <= 15 else 1
    v_pools = [ctx.enter_context(
        tc.tile_pool(name=f"v{g}", bufs=vbufs, space="PSUM"))
        for g in (0, 1)]
    z_psum = ctx.enter_context(tc.tile_pool(name="zp", bufs=1, space="PSUM"))

    # weights as three (128,128) lhsT tiles laid side by side on partitions 0..127
    w_t = consts.tile([128, 3, 128], BF16, name="wt3")
    nc.sync.dma_start(w_t[:, 0, :], wts[0])
    nc.sync.dma_start(w_t[:, 1, :], wts[1])
    nc.sync.dma_start(w_t[:, 2, :], wts[2])
    WF, WB, WZ = w_t[:, 0, :], w_t[:, 1, :], w_t[:, 2, :]

    cols_t = consts.tile([128, 2], F32, name="cols_t")
    nc.sync.dma_start(cols_t[:, :], cols.rearrange("c p -> p c"))
    PIP, TAU = cols_t[:, 0:1], cols_t[:, 1:2]
    warm = consts.tile([128, 1], F32, name="warm")
    nc.scalar.copy(warm[:, :], cols_t[:, 0:1])
    dwarm = consts.tile([128, 1], F32, name="dwarm")
    nc.vector.tensor_copy(dwarm[:, :], cols_t[:, 0:1])
    sd_t = consts.tile([1, 2, 3], F32, name="sd_t")
    nc.vector.memset(sd_t[:, :, :], 0.0)

    # z history: (128, NCH, 32, 2) slots: [0]=event z / recip(neg), [1]=stitch pos
    zh = [consts.tile([128, NCH, 32, 2], F32, name=f"zh{g}") for g in (0, 1)]
    for g in (0, 1):
        nc.vector.memset(zh[g][:, :, :, :], 1.0)

    csplit = [(c0, min(c0 + 2, NCH)) for c0 in range(0, NCH, 2)]

    PDT = lp.dtype

    def load_chunk(g, i):
        if s[g] is not None:
            # absorber: lets ACT observe the DVE tick that retires the chunk
            # buffer being reused, so the exps below don't carry a 3rd wait
            nc.scalar.copy(sd_t[0:1, g, 0:1], sd_t[0:1, g, 1:2])
        t_ = pch_pools[g].tile([128, NCH, 32, TCc], PDT, name="pch", tag="pch")
        for j, (c0, c1) in enumerate(csplit):
            # one DMA covers both partition halves -> the exp waits one sem
            eng = nc.sync if (g + j) % 2 == 0 else nc.gpsimd
            eng.dma_start(
                t_[:, c0:c1, :, :],
                lp[g, i, c0:c1].rearrange("c gs k b t -> (gs k) c b t"),
            )
            # CHAT is pre-subtracted on the host; bias=0 keeps deps minimal
            nc.scalar.activation(t_[:, c0:c1, :, :], t_[:, c0:c1, :, :],
                                 AFT.Exp)
        return t_

    s = [None, None]
    pch = [load_chunk(g, 0) for g in (0, 1)]

    # ---- slot 0: inits (read position t=0 of chunk 0) ----
    for g in (0, 1):
        s0 = s_pools[g].tile([128, NCH, 32], BF16, name="s", tag="s")
        p0 = pch[g][:, :, :, 0]        # (128, NCH, 32) position 0 slices
        nc.vector.tensor_scalar_mul(s0[:, 0, :], p0[:, 0, :], PIP)
        nc.vector.tensor_scalar_mul(s0[:, 1:H, :], p0[:, 1:H, :], TAU)
        nc.vector.tensor_copy(s0[:, H:NCH, :], p0[:, H:NCH, :])
        s[g] = s0

    def pe_absorb(t_dep):
        # ldweights reads the dependency tile: PE observes the producer's
        # tick without any tracked write, so following matmuls carry at most
        # one sync wait (the hardware MM limit)
        nc.tensor.ldweights(weights=t_dep)

    def zmm_all(g, s_cur, c0=0, c1=None):
        c1 = NCH if c1 is None else c1
        zb = z_psum.tile([128, NCH, 32], F32, name="zb", tag="zb")
        pe_absorb(s_cur[:, 0, 0:1])
        for c in range(c0, c1):
            nc.tensor.matmul(zb[:, c, :], lhsT=WZ, rhs=s_cur[:, c, :],
                             start=True, stop=True)
        return zb

    # ---- main slot loop (prefetch next chunk before stepping current) ----
    nxt = None
    for sig in range(1, S):
        i = sig // TCc
        if sig % TCc == 1 and i + 1 < n_chunks:
            nxt = [load_chunk(g, i + 1) for g in (0, 1)]
        if sig % TCc == 0 and i > 0:
            pch = nxt
        for g in (0, 1):
            v = v_pools[g].tile([128, NCH, 32], F32, name="v", tag="v")
            pe_absorb(s[g][:, 0, 0:1])
            for c in range(NCH):
                nc.tensor.matmul(v[:, c, :], lhsT=(WF if c < H else WB),
                                 rhs=s[g][:, c, :], start=True, stop=True)
            s_new = s_pools[g].tile([128, NCH, 32], BF16, name="s", tag="s")
            nc.vector.tensor_mul(s_new[:, :, :], v[:, :, :],
                                 pch[g][:, :, :, sig % TCc])
            s[g] = s_new

            if sig % TCc == TCc - 1:
                # retire marker: rides the s chain so its tick dominates every
                # reader of the finishing chunk (ACT absorber reads it later)
                nc.vector.tensor_copy(sd_t[0:1, g, 1:2], s[g][0:1, 0, 0:1])
                if nxt is not None:
                    # DVE observes the next chunk's exps before the boundary
                    # multiply so that multiply needs no 3rd sem wait
                    nc.vector.tensor_copy(sd_t[0:1, g, 2:3],
                                          nxt[g][0:1, NCH - 1, 0, 0:1])

            if sig % r == r - 1 and sig != S - 1:
                zb = zmm_all(g, s[g])
                rinv = small.tile([128, NCH, 32], F32, name="rinv", tag="rinv")
                nc.vector.reciprocal(rinv[:, :, :], zb[:, :, :])
                nc.vector.tensor_copy(zh[g][:, 0:H, :, 0], zb[:, 0:H, :])
                s_rn = s_pools[g].tile([128, NCH, 32], BF16, name="s", tag="s")
                nc.vector.tensor_mul(s_rn[:, :, :], s[g][:, :, :], rinv[:, :, :])
                s[g] = s_rn

    # ---- stitch ----
    for g in (0, 1):
        # bare backward matmuls: w_h = Tp @ w~_last
        wv = v_pools[g].tile([128, NCH, 32], F32, name="v", tag="v")
        pe_absorb(s[g][:, 0, 0:1])
        for c in range(H, NCH):
            nc.tensor.matmul(wv[:, c, :], lhsT=WB, rhs=s[g][:, c, :],
                             start=True, stop=True)
        wfin = small.tile([128, H - 1, 32], BF16, name="wfin", tag="wfin")
        nc.vector.tensor_copy(wfin[:, :, :], wv[:, H:NCH, :])
        # dots: w_h * x_{h-1}  (x-chain slices are exactly c=0..H-2)
        dprod = small.tile([128, H - 1, 32], BF16, name="dprod", tag="dprod")
        nc.vector.tensor_mul(dprod[:, :, :], wfin[:, :, :], s[g][:, 0:H - 1, :])
        # pos terms: colsum(dprod) for c=0..H-2, colsum(y_{H-1}) at c=H-1
        zp = z_psum.tile([128, NCH, 32], F32, name="zb", tag="zb")
        pe_absorb(dprod[:, 0, 0:1])
        for c in range(H - 1):
            nc.tensor.matmul(zp[:, c, :], lhsT=WZ, rhs=dprod[:, c, :],
                             start=True, stop=True)
        nc.tensor.matmul(zp[:, H - 1, :], lhsT=WZ, rhs=s[g][:, H - 1, :],
                         start=True, stop=True)
        # neg terms: colsum(w_h) -> store reciprocal (ln(1/x) = -ln x)
        for c in range(H, NCH):
            nc.tensor.matmul(zp[:, c, :], lhsT=WZ, rhs=wfin[:, c - H, :],
                             start=True, stop=True)
        nc.vector.tensor_copy(zh[g][:, 0:H, :, 1], zp[:, 0:H, :])
        nc.vector.reciprocal(zh[g][:, H:NCH, :, 0], zp[:, H:NCH, :])
        # ll = sum over (c, slot) of ln(zh)
        lnh = small.tile([128, NCH, 32, 2], F32, name="lnh", tag="lnh")
        nc.scalar.activation(lnh[:, :, :, :], zh[g][:, :, :, :], AFT.Ln)
        ll = small.tile([128, 32], F32, name="ll", tag="ll")
        lnh_bcs = bass.AP(tensor=lnh.tensor, offset=lnh.offset,
                          ap=[lnh.ap[0], [2, 32], [64, NCH], [1, 2]])
        nc.vector.tensor_reduce(ll[:, :], lnh_bcs, mybir.AxisListType.XY,
                                mybir.AluOpType.add)
        nc.sync.dma_start(out_ll[g, :, :], ll[:, :])


# ---------------- host side ----------------

def _log_softmax(x, axis):
    x = np.asarray(x, np.float64)
    m = x.max(axis=axis, keepdims=True)
    return x - m - np.log(np.exp(x - m).sum(axis=axis, keepdims=True))


def prep_inputs(log_pdf, pi, T, ns, h_seg=8, tc_chunk=16, n_cores=8,
                lp_fp16=True):
    Kd, N = log_pdf.shape
    b_total = N // ns
    b_core = b_total // n_cores
    S = ns // h_seg
    NCH = 2 * h_seg - 1
    n_chunks = S // tc_chunk

    logT = _log_softmax(T, 1)
    Tp = np.exp(logT)
    logpi = _log_softmax(pi, 0)

    wf = np.zeros((128, 128), np.float64)
    wf[:64, :64] = Tp; wf[64:, 64:] = Tp
    wb = np.zeros((128, 128), np.float64)
    wb[:64, :64] = Tp.T; wb[64:, 64:] = Tp.T
    wz = np.zeros((128, 128), np.float64)
    wz[:64, :64] = 1.0; wz[64:, 64:] = 1.0
    wts = np.stack([wf, wb, wz]).astype(ml_dtypes.bfloat16)

    pip = np.exp(logpi)                      # s0 init scalar
    tau = Tp.sum(axis=0)                     # Tp^T @ 1 per state
    cols = np.stack([np.concatenate([pip, pip]),
                     np.concatenate([tau, tau])]).astype(np.float32)  # (2,128)

    # P-position index maps per chain: pos p in [0,S) -> global t
    tmap = np.empty((NCH, S), np.int64)
    tmap[0] = np.arange(S)                                   # seg0
    for h in range(1, h_seg):
        tmap[h] = h * S + np.arange(S)                       # fwd h
        tmap[h_seg + h - 1] = (h + 1) * S - 1 - np.arange(S)  # bwd h (reversed)

    in_maps = []
    for core in range(n_cores):
        lp3 = log_pdf[:, core * b_core * ns: (core + 1) * b_core * ns]
        lp3 = np.asarray(lp3, np.float32).reshape(Kd, b_core, ns)  # [k,b,t]
        # gather [k, b, c, pos]
        gat = lp3[:, :, tmap.reshape(-1)].reshape(Kd, b_core, NCH, S) - CHAT
        # want [gp][i][gs][c][k][b32][t] ; b = 64*gp + 32*gs + b32
        gat = gat.reshape(Kd, 2, 2, 32, NCH, n_chunks, tc_chunk)
        pk = np.ascontiguousarray(gat.transpose(1, 5, 4, 2, 0, 3, 6),
                                  dtype=np.float16 if lp_fp16 else np.float32)
        in_maps.append({"lp": pk, "wts": wts, "cols": cols})
    return in_maps


def finish_output(results, ns):
    total = 0.0
    for res in results:
        ll = np.asarray(res["ll"], np.float64)  # (2,128,32)
        for g in (0, 1):
            for gs in (0, 1):
                total += (ll[g, 64 * gs, :] + ns * CHAT).sum()
    return np.float32(total)


# ---------------- harness entry point ----------------

_CACHED = {}


def _get_nc():
    if "nc" not in _CACHED:
        _CACHED["nc"] = build_nc(ns=1024, h_seg=8, tc_chunk=16, r=64)
    return _CACHED["nc"]


def kernel(log_pdf, pi, T, samples_per_sequence):
    """Full unsharded inputs -> full output (scalar f32), computed on 8
    TRN2 NeuronCores via the time-segmented scaled-forward kernel."""
    from concourse.bass_utils import run_bass_kernel_spmd

    ns = int(samples_per_sequence)
    assert log_pdf.shape == (64, 1048576) and ns == 1024, (
        "kernel is specialized to K=64, N=1048576, Ns=1024"
    )
    nc = _get_nc()
    in_maps = prep_inputs(np.asarray(log_pdf, np.float32),
                          np.asarray(pi, np.float32),
                          np.asarray(T, np.float32),
                          ns, h_seg=8, tc_chunk=16, n_cores=8)
    res = run_bass_kernel_spmd(nc, in_maps, core_ids=list(range(8)))
    return np.asarray(finish_output(res.results, ns), np.float32)

